# revision 4
# baseline (speedup 1.0000x reference)
"""GroupedRouter Bass kernel for 8 TRN2 NeuronCores.

Reference computation (per batch b, head h):
    q = x @ Wq, k = x @ Wk           (heads of dim 128)
    scores = q k^T / sqrt(128)       [N, N]
    group max over 8 key groups of 128, keep top-2 groups, softmax over kept.

Sharding: core c -> batch b = c//2, head half hh = c%2 (8 heads per core).
Each core computes out[b, :, hh*8:(hh+1)*8, :] locally: fully data-parallel,
no collectives.

Precision strategy: all matmuls run at bf16 rate using an error-compensated
bf16x2 split (v = v1 + v2 with v1 = bf16(v), v2 = bf16(v - v1)); products
keep ~2^-16 relative accuracy via three accumulating passes
(a1*b1 + a1*b2 + a2*b1) into fp32 PSUM. x and W are split host-side (same
total bytes as fp32); x is also transposed host-side into D-major layout, so
the kernel needs no on-chip transpose. q/k are re-split on-chip at the
PSUM->SBUF copyback.

Per-core pipeline:
  1) per head: stream Wq/Wk head slices (bf16 pair), 3-pass matmul ->
     qT,kT [128(dh), 1024(tok)] bf16 pairs (q scaled by 1/sqrt(128)).
  2) per (head, 128-query chunk): 3-pass scores -> PSUM [128,1024] fp32;
     grouped max (DVE reduce over [128,8,128]); top-2 threshold; per-group
     bias = -rowmax (kept) / -BIG (masked); ACT exp with bias + accumulated
     row-sum; reciprocal; GPSIMD normalize; DMA out.
"""
import numpy as np
import orjson
import ml_dtypes

import concourse.bass as bass
import concourse.mybir as mybir
from concourse.tile import TileContext
from concourse.bass_utils import run_bass_kernel_spmd
from concourse.bass import ts, ds

B, N, D = 4, 1024, 2048
H, DH = 16, 128
G = 8
GSIZE = N // G          # 128
NCORES = 8
HPC = H // 2            # heads per core
SCALE = float(1.0 / np.sqrt(DH))
BIG = 30000.0

f32 = mybir.dt.float32
bf16 = mybir.dt.bfloat16
Alu = mybir.AluOpType
Act = mybir.ActivationFunctionType
AxX = mybir.AxisListType.X

# ---------------------------------------------------------------------------
# BIR sync-wait legalizer: walrus for cayman accepts only one sync-wait
# command per instruction; Tile attaches one per dependency. Hoist the excess
# onto standalone EventSemaphore instructions immediately before the target
# (engine queues are FIFO, so blocking semantics are unchanged).
# ---------------------------------------------------------------------------

def _legalize_bir(bir: dict) -> dict:
    ctr = 0
    for fn in bir["functions"]:
        for bb in fn["blocks"]:
            insts = bb.get("instructions")
            if not insts:
                continue
            out = []
            for ins in insts:
                si = ins.get("sync_info")
                waits = (si or {}).get("on_wait") or []
                if len(waits) > 1:
                    for w in waits[:-1]:
                        ctr += 1
                        out.append({
                            "engine": ins["engine"],
                            "ins": [],
                            "outs": [],
                            "name": f"legwait-{ctr}",
                            "opcode": "EventSemaphore",
                            "sync_info": {"on_update": [], "on_wait": [w]},
                        })
                    si["on_wait"] = waits[-1:]
                out.append(ins)
            bb["instructions"] = out
    return bir


def _install_legalizer(nc):
    orig = nc.to_json_bytes

    def to_json_bytes():
        return orjson.dumps(_legalize_bir(orjson.loads(orig())))

    nc.to_json_bytes = to_json_bytes


# ---------------------------------------------------------------------------
# Kernel build (one SPMD program; per-core differences live in the input data)
# ---------------------------------------------------------------------------

def _build():
    nc = bass.Bass()
    # x[b] transposed host-side to D-major, split into bf16 hi/lo planes.
    xt1 = nc.declare_dram_parameter("xt1", [D, N], bf16, isOutput=False)
    xt2 = nc.declare_dram_parameter("xt2", [D, N], bf16, isOutput=False)
    wq1 = nc.declare_dram_parameter("wq1", [D, HPC * DH], bf16, isOutput=False)
    wq2 = nc.declare_dram_parameter("wq2", [D, HPC * DH], bf16, isOutput=False)
    wk1 = nc.declare_dram_parameter("wk1", [D, HPC * DH], bf16, isOutput=False)
    wk2 = nc.declare_dram_parameter("wk2", [D, HPC * DH], bf16, isOutput=False)
    out = nc.declare_dram_parameter("out", [N, HPC * N], f32, isOutput=True)

    nk = D // 128  # 16 contraction chunks
    xt1_3 = xt1.rearrange("(kc p) t -> p kc t", p=128)
    xt2_3 = xt2.rearrange("(kc p) t -> p kc t", p=128)
    w3 = [w.rearrange("(kc p) hd -> p kc hd", p=128)
          for w in (wq1, wq2, wk1, wk2)]

    with TileContext(nc) as tc:
        with tc.tile_pool(name="const", bufs=1) as cpool, \
             tc.tile_pool(name="xT", bufs=1) as xtp:
            negbig = cpool.tile([128, G], f32)
            nc.vector.memset(negbig[:], -BIG)

            # resident x planes: [128, kc*tok] bf16, 32KB/partition each
            xa = xtp.tile([128, nk * N], bf16, name="xa", tag="xa")
            xb_ = xtp.tile([128, nk * N], bf16, name="xb", tag="xb")
            nc.sync.dma_start(
                out=xa[:].rearrange("p (kc t) -> p kc t", t=N), in_=xt1_3[:])
            nc.sync.dma_start(
                out=xb_[:].rearrange("p (kc t) -> p kc t", t=N), in_=xt2_3[:])

            def xA(kc):
                return xa[:, ds(kc * N, N)]

            def xB(kc):
                return xb_[:, ds(kc * N, N)]

            with tc.tile_pool(name="w", bufs=2) as wpool, \
                 tc.tile_pool(name="qk", bufs=2) as qkp, \
                 tc.tile_pool(name="psp", bufs=2, space="PSUM") as psp, \
                 tc.tile_pool(name="pss", bufs=2, space="PSUM") as pss, \
                 tc.tile_pool(name="ep", bufs=4) as ep, \
                 tc.tile_pool(name="outp", bufs=3) as outp:
                for h in range(HPC):
                    # --- projections: 3-pass bf16x2 ---
                    qk_pair = []
                    for wi, (whi3, wlo3) in enumerate(
                            ((w3[0], w3[1]), (w3[2], w3[3]))):
                        whi = wpool.tile([128, nk * 128], bf16, tag="whi")
                        wlo = wpool.tile([128, nk * 128], bf16, tag="wlo")
                        nc.sync.dma_start(
                            out=whi[:].rearrange("p (kc hd) -> p kc hd", hd=128),
                            in_=whi3[:, :, ts(h, 128)])
                        nc.sync.dma_start(
                            out=wlo[:].rearrange("p (kc hd) -> p kc hd", hd=128),
                            in_=wlo3[:, :, ts(h, 128)])
                        ps = psp.tile([128, N], f32, tag="pp")
                        for half in range(2):
                            sl = ds(half * 512, 512)
                            passes = [(whi, xA), (wlo, xA), (whi, xB)]
                            for pi, (wt, xf) in enumerate(passes):
                                for kc in range(nk):
                                    nc.tensor.matmul(
                                        ps[:, sl], wt[:, ts(kc, 128)],
                                        xf(kc)[:, sl],
                                        start=(pi == 0 and kc == 0),
                                        stop=(pi == 2 and kc == nk - 1))
                        # copyback with bf16x2 re-split (scale q by 1/sqrt(dh))
                        s = SCALE if wi == 0 else 1.0
                        hi = qkp.tile([128, N], bf16, tag=f"hi{wi}")
                        lo = qkp.tile([128, N], bf16, tag=f"lo{wi}")
                        nc.scalar.activation(hi[:], ps[:], Act.Copy,
                                             bias=0.0, scale=s)
                        nc.vector.scalar_tensor_tensor(
                            lo[:], ps[:], s, hi[:],
                            op0=Alu.mult, op1=Alu.subtract)
                        qk_pair.append((hi, lo))
                    (q1, q2), (k1, k2) = qk_pair

                    # --- scores + grouped softmax per 128-query chunk ---
                    for qc in range(8):
                        sps = pss.tile([128, N], f32, tag="ss")
                        for half in range(2):
                            sl = ds(half * 512, 512)
                            passes = [(q1, k1), (q1, k2), (q2, k1)]
                            for pi, (qa, kb) in enumerate(passes):
                                nc.tensor.matmul(
                                    sps[:, sl], qa[:, ts(qc, 128)], kb[:, sl],
                                    start=(pi == 0), stop=(pi == 2))

                        gs = ep.tile([128, G], f32, tag="gs")
                        nc.vector.tensor_reduce(
                            gs[:], sps[:].rearrange("p (g j) -> p g j", j=GSIZE),
                            axis=AxX, op=Alu.max)
                        m1 = ep.tile([128, 1], f32, tag="m1")
                        nc.vector.tensor_reduce(m1[:], gs[:], axis=AxX, op=Alu.max)
                        eq = ep.tile([128, G], f32, tag="eq")
                        nc.vector.tensor_scalar(eq[:], gs[:], m1[:], None,
                                                op0=Alu.is_ge)
                        gs2 = ep.tile([128, G], f32, tag="gs2")
                        nc.vector.scalar_tensor_tensor(
                            gs2[:], eq[:], -BIG, gs[:],
                            op0=Alu.mult, op1=Alu.add)
                        m2 = ep.tile([128, 1], f32, tag="m2")
                        nc.vector.tensor_reduce(m2[:], gs2[:], axis=AxX, op=Alu.max)
                        cmp = ep.tile([128, G], f32, tag="cmp")
                        nc.vector.tensor_scalar(cmp[:], gs[:], m2[:], None,
                                                op0=Alu.is_ge)
                        m1b = ep.tile([128, 1], f32, tag="m1b")
                        nc.vector.tensor_scalar(m1b[:], m1[:], -1.0, BIG,
                                                op0=Alu.mult, op1=Alu.add)
                        bias = ep.tile([128, G], f32, tag="bias")
                        nc.vector.scalar_tensor_tensor(
                            bias[:], cmp[:], m1b[:], negbig[:],
                            op0=Alu.mult, op1=Alu.add)

                        acc = ep.tile([128, G], f32, tag="acc")
                        eo = outp.tile([128, N], f32, tag="eo")
                        for g in range(G):
                            nc.scalar.activation(
                                eo[:, ts(g, GSIZE)], sps[:, ts(g, GSIZE)],
                                Act.Exp, bias=bias[:, g:g + 1], scale=1.0,
                                accum_out=acc[:, g:g + 1])
                        rs = ep.tile([128, 1], f32, tag="rs")
                        nc.vector.tensor_reduce(rs[:], acc[:], axis=AxX, op=Alu.add)
                        rc = ep.tile([128, 1], f32, tag="rc")
                        nc.vector.reciprocal(rc[:], rs[:])
                        nc.gpsimd.tensor_scalar_mul(eo[:], eo[:], rc[:])
                        nc.sync.dma_start(
                            out=out[ts(qc, 128), ds(h * N, N)], in_=eo[:])

    _install_legalizer(nc)
    return nc


_NC_CACHE = {}


def _get_nc():
    if "nc" not in _NC_CACHE:
        _NC_CACHE["nc"] = _build()
    return _NC_CACHE["nc"]


def _bf16_pair(a):
    hi = a.astype(ml_dtypes.bfloat16)
    lo = (a - hi.astype(np.float32)).astype(ml_dtypes.bfloat16)
    return hi, lo


def _in_maps(x, Wq, Wk):
    maps = []
    for c in range(NCORES):
        b, hh = c // 2, c % 2
        sl = slice(hh * HPC * DH, (hh + 1) * HPC * DH)
        xt1, xt2 = _bf16_pair(np.ascontiguousarray(x[b].T))
        wq1, wq2 = _bf16_pair(np.ascontiguousarray(Wq[:, sl]))
        wk1, wk2 = _bf16_pair(np.ascontiguousarray(Wk[:, sl]))
        maps.append({"xt1": xt1, "xt2": xt2, "wq1": wq1, "wq2": wq2,
                     "wk1": wk1, "wk2": wk2})
    return maps


def kernel(x, Wq, Wk, **kwargs):
    x = np.asarray(x, dtype=np.float32)
    Wq = np.asarray(Wq, dtype=np.float32)
    Wk = np.asarray(Wk, dtype=np.float32)
    nc = _get_nc()
    res = run_bass_kernel_spmd(nc, _in_maps(x, Wq, Wk),
                               core_ids=list(range(NCORES)))
    full = np.empty((B, N, H, N), dtype=np.float32)
    for c in range(NCORES):
        b, hh = c // 2, c % 2
        full[b, :, hh * HPC:(hh + 1) * HPC, :] = (
            res.results[c]["out"].reshape(N, HPC, N))
    return full


# revision 5
# speedup vs baseline: 2.4575x; 2.4575x over previous
"""GroupedRouter Bass kernel for 8 TRN2 NeuronCores.

Reference computation (per batch b, head h):
    q = x @ Wq, k = x @ Wk           (heads of dim 128)
    scores = q k^T / sqrt(128)       [N, N]
    group max over 8 key groups of 128, keep top-2 groups, softmax over kept.

Sharding: core c -> batch b = c//2, head half hh = c%2 (8 heads per core).
Each core computes out[b, :, hh*8:(hh+1)*8, :] locally: fully data-parallel,
no collectives.

Precision strategy: all matmuls run at bf16 rate using an error-compensated
bf16x2 split (v = v1 + v2 with v1 = bf16(v), v2 = bf16(v - v1)); products
keep ~2^-16 relative accuracy via three accumulating passes
(a1*b1 + a1*b2 + a2*b1) into fp32 PSUM. x and W are split host-side (same
total bytes as fp32); x is also transposed host-side into D-major layout, so
the kernel needs no on-chip transpose. q/k are re-split on-chip at the
PSUM->SBUF copyback.

Per-core pipeline:
  1) per head: stream Wq/Wk head slices (bf16 pair), 3-pass matmul ->
     qT,kT [128(dh), 1024(tok)] bf16 pairs (q scaled by 1/sqrt(128)).
  2) per (head, 128-query chunk): 3-pass scores -> PSUM [128,1024] fp32;
     grouped max (DVE reduce over [128,8,128]); top-2 threshold; per-group
     bias = -rowmax (kept) / -BIG (masked); ACT exp with bias + accumulated
     row-sum; reciprocal; GPSIMD normalize; DMA out.
"""
import numpy as np
import orjson
import ml_dtypes

import concourse.bass as bass
import concourse.mybir as mybir
from concourse.tile import TileContext
from concourse.bass_utils import run_bass_kernel_spmd
from concourse.bass import ts, ds

B, N, D = 4, 1024, 2048
H, DH = 16, 128
G = 8
GSIZE = N // G          # 128
NCORES = 8
HPC = H // 2            # heads per core
SCALE = float(1.0 / np.sqrt(DH))
BIG = 30000.0

f32 = mybir.dt.float32
bf16 = mybir.dt.bfloat16
Alu = mybir.AluOpType
Act = mybir.ActivationFunctionType
AxX = mybir.AxisListType.X

# ---------------------------------------------------------------------------
# BIR sync-wait legalizer: walrus for cayman accepts only one sync-wait
# command per instruction; Tile attaches one per dependency. Hoist the excess
# onto standalone EventSemaphore instructions immediately before the target
# (engine queues are FIFO, so blocking semantics are unchanged).
# ---------------------------------------------------------------------------

def _legalize_bir(bir: dict) -> dict:
    ctr = 0
    for fn in bir["functions"]:
        for bb in fn["blocks"]:
            insts = bb.get("instructions")
            if not insts:
                continue
            out = []
            for ins in insts:
                si = ins.get("sync_info")
                waits = (si or {}).get("on_wait") or []
                if len(waits) > 1:
                    for w in waits[:-1]:
                        ctr += 1
                        out.append({
                            "engine": ins["engine"],
                            "ins": [],
                            "outs": [],
                            "name": f"legwait-{ctr}",
                            "opcode": "EventSemaphore",
                            "sync_info": {"on_update": [], "on_wait": [w]},
                        })
                    si["on_wait"] = waits[-1:]
                out.append(ins)
            bb["instructions"] = out
    return bir


def _install_legalizer(nc):
    orig = nc.to_json_bytes

    def to_json_bytes():
        return orjson.dumps(_legalize_bir(orjson.loads(orig())))

    nc.to_json_bytes = to_json_bytes


# ---------------------------------------------------------------------------
# Kernel build (one SPMD program; per-core differences live in the input data)
# ---------------------------------------------------------------------------

def _build():
    nc = bass.Bass()
    # x[b] transposed host-side to D-major, split into bf16 hi/lo planes.
    xt1 = nc.declare_dram_parameter("xt1", [D, N], bf16, isOutput=False)
    xt2 = nc.declare_dram_parameter("xt2", [D, N], bf16, isOutput=False)
    wq1 = nc.declare_dram_parameter("wq1", [D, HPC * DH], bf16, isOutput=False)
    wq2 = nc.declare_dram_parameter("wq2", [D, HPC * DH], bf16, isOutput=False)
    wk1 = nc.declare_dram_parameter("wk1", [D, HPC * DH], bf16, isOutput=False)
    wk2 = nc.declare_dram_parameter("wk2", [D, HPC * DH], bf16, isOutput=False)
    out = nc.declare_dram_parameter("out", [N, HPC * N], f32, isOutput=True)

    nk = D // 128  # 16 contraction chunks
    xt1_3 = xt1.rearrange("(kc p) t -> p kc t", p=128)
    xt2_3 = xt2.rearrange("(kc p) t -> p kc t", p=128)
    w3 = [w.rearrange("(kc p) hd -> p kc hd", p=128)
          for w in (wq1, wq2, wk1, wk2)]

    with TileContext(nc) as tc:
        with tc.tile_pool(name="const", bufs=1) as cpool, \
             tc.tile_pool(name="xT", bufs=1) as xtp:
            negbig = cpool.tile([128, G], f32)
            nc.vector.memset(negbig[:], -BIG)

            # resident x planes: [128, kc*tok] bf16, 32KB/partition each
            xa = xtp.tile([128, nk * N], bf16, name="xa", tag="xa")
            xb_ = xtp.tile([128, nk * N], bf16, name="xb", tag="xb")
            nc.sync.dma_start(
                out=xa[:].rearrange("p (kc t) -> p kc t", t=N), in_=xt1_3[:])
            nc.sync.dma_start(
                out=xb_[:].rearrange("p (kc t) -> p kc t", t=N), in_=xt2_3[:])

            def xA(kc):
                return xa[:, ds(kc * N, N)]

            def xB(kc):
                return xb_[:, ds(kc * N, N)]

            with tc.tile_pool(name="w", bufs=2) as wpool, \
                 tc.tile_pool(name="qk", bufs=2) as qkp, \
                 tc.tile_pool(name="psp", bufs=2, space="PSUM") as psp, \
                 tc.tile_pool(name="pss", bufs=2, space="PSUM") as pss, \
                 tc.tile_pool(name="ep", bufs=4) as ep, \
                 tc.tile_pool(name="outp", bufs=3) as outp:
                for h in range(HPC):
                    # --- projections: 3-pass bf16x2 ---
                    qk_pair = []
                    for wi, (whi3, wlo3) in enumerate(
                            ((w3[0], w3[1]), (w3[2], w3[3]))):
                        whi = wpool.tile([128, nk * 128], bf16, tag="whi")
                        wlo = wpool.tile([128, nk * 128], bf16, tag="wlo")
                        nc.sync.dma_start(
                            out=whi[:].rearrange("p (kc hd) -> p kc hd", hd=128),
                            in_=whi3[:, :, ts(h, 128)])
                        nc.sync.dma_start(
                            out=wlo[:].rearrange("p (kc hd) -> p kc hd", hd=128),
                            in_=wlo3[:, :, ts(h, 128)])
                        ps = psp.tile([128, N], f32, tag="pp")
                        for half in range(2):
                            sl = ds(half * 512, 512)
                            passes = [(whi, xA), (wlo, xA), (whi, xB)]
                            for pi, (wt, xf) in enumerate(passes):
                                for kc in range(nk):
                                    nc.tensor.matmul(
                                        ps[:, sl], wt[:, ts(kc, 128)],
                                        xf(kc)[:, sl],
                                        start=(pi == 0 and kc == 0),
                                        stop=(pi == 2 and kc == nk - 1))
                        # copyback with bf16x2 re-split (scale q by 1/sqrt(dh))
                        s = SCALE if wi == 0 else 1.0
                        hi = qkp.tile([128, N], bf16, tag=f"hi{wi}")
                        lo = qkp.tile([128, N], bf16, tag=f"lo{wi}")
                        nc.scalar.activation(hi[:], ps[:], Act.Copy,
                                             bias=0.0, scale=s)
                        nc.vector.scalar_tensor_tensor(
                            lo[:], ps[:], s, hi[:],
                            op0=Alu.mult, op1=Alu.subtract)
                        qk_pair.append((hi, lo))
                    (q1, q2), (k1, k2) = qk_pair

                    # --- scores + grouped softmax per 128-query chunk ---
                    for qc in range(8):
                        sps = pss.tile([128, N], f32, tag="ss")
                        for half in range(2):
                            sl = ds(half * 512, 512)
                            passes = [(q1, k1), (q1, k2), (q2, k1)]
                            for pi, (qa, kb) in enumerate(passes):
                                nc.tensor.matmul(
                                    sps[:, sl], qa[:, ts(qc, 128)], kb[:, sl],
                                    start=(pi == 0), stop=(pi == 2))

                        gs = ep.tile([128, G], f32, tag="gs")
                        nc.vector.tensor_reduce(
                            gs[:], sps[:].rearrange("p (g j) -> p g j", j=GSIZE),
                            axis=AxX, op=Alu.max)
                        m1 = ep.tile([128, 1], f32, tag="m1")
                        nc.vector.tensor_reduce(m1[:], gs[:], axis=AxX, op=Alu.max)
                        eq = ep.tile([128, G], f32, tag="eq")
                        nc.vector.tensor_tensor(
                            eq[:], gs[:], m1[:].broadcast_to((128, G)),
                            op=Alu.is_ge)
                        gs2 = ep.tile([128, G], f32, tag="gs2")
                        nc.vector.scalar_tensor_tensor(
                            gs2[:], eq[:], -BIG, gs[:],
                            op0=Alu.mult, op1=Alu.add)
                        m2 = ep.tile([128, 1], f32, tag="m2")
                        nc.vector.tensor_reduce(m2[:], gs2[:], axis=AxX, op=Alu.max)
                        cmp = ep.tile([128, G], f32, tag="cmp")
                        nc.vector.tensor_tensor(
                            cmp[:], gs[:], m2[:].broadcast_to((128, G)),
                            op=Alu.is_ge)
                        m1b = ep.tile([128, 1], f32, tag="m1b")
                        nc.vector.tensor_reduce(m1b[:], gs[:], axis=AxX,
                                                op=Alu.max, negate=True)
                        # bias = cmp * (BIG + (-m1)) - BIG  (kept: -m1, masked: -BIG)
                        m1c = ep.tile([128, 1], f32, tag="m1c")
                        nc.vector.scalar_tensor_tensor(
                            m1c[:], m1b[:], BIG, m1b[:],
                            op0=Alu.add, op1=Alu.bypass)
                        bias = ep.tile([128, G], f32, tag="bias")
                        nc.vector.scalar_tensor_tensor(
                            bias[:], cmp[:], -BIG,
                            m1c[:].broadcast_to((128, G)),
                            op0=Alu.bypass, op1=Alu.mult)
                        nc.vector.tensor_scalar_add(bias[:], bias[:], -BIG)

                        acc = ep.tile([128, G], f32, tag="acc")
                        eo = outp.tile([128, N], f32, tag="eo")
                        for g in range(G):
                            nc.scalar.activation(
                                eo[:, ts(g, GSIZE)], sps[:, ts(g, GSIZE)],
                                Act.Exp, bias=bias[:, g:g + 1], scale=1.0,
                                accum_out=acc[:, g:g + 1])
                        rs = ep.tile([128, 1], f32, tag="rs")
                        nc.vector.tensor_reduce(rs[:], acc[:], axis=AxX, op=Alu.add)
                        rc = ep.tile([128, 1], f32, tag="rc")
                        nc.vector.reciprocal(rc[:], rs[:])
                        nc.scalar.activation(eo[:], eo[:], Act.Copy,
                                             bias=0.0, scale=rc[:])
                        nc.sync.dma_start(
                            out=out[ts(qc, 128), ds(h * N, N)], in_=eo[:])

    _install_legalizer(nc)
    return nc


_NC_CACHE = {}


def _get_nc():
    if "nc" not in _NC_CACHE:
        _NC_CACHE["nc"] = _build()
    return _NC_CACHE["nc"]


def _bf16_pair(a):
    hi = a.astype(ml_dtypes.bfloat16)
    lo = (a - hi.astype(np.float32)).astype(ml_dtypes.bfloat16)
    return hi, lo


def _in_maps(x, Wq, Wk):
    maps = []
    for c in range(NCORES):
        b, hh = c // 2, c % 2
        sl = slice(hh * HPC * DH, (hh + 1) * HPC * DH)
        xt1, xt2 = _bf16_pair(np.ascontiguousarray(x[b].T))
        wq1, wq2 = _bf16_pair(np.ascontiguousarray(Wq[:, sl]))
        wk1, wk2 = _bf16_pair(np.ascontiguousarray(Wk[:, sl]))
        maps.append({"xt1": xt1, "xt2": xt2, "wq1": wq1, "wq2": wq2,
                     "wk1": wk1, "wk2": wk2})
    return maps


def kernel(x, Wq, Wk, **kwargs):
    x = np.asarray(x, dtype=np.float32)
    Wq = np.asarray(Wq, dtype=np.float32)
    Wk = np.asarray(Wk, dtype=np.float32)
    nc = _get_nc()
    res = run_bass_kernel_spmd(nc, _in_maps(x, Wq, Wk),
                               core_ids=list(range(NCORES)))
    full = np.empty((B, N, H, N), dtype=np.float32)
    for c in range(NCORES):
        b, hh = c // 2, c % 2
        full[b, :, hh * HPC:(hh + 1) * HPC, :] = (
            res.results[c]["out"].reshape(N, HPC, N))
    return full
